# revision 13
# baseline (speedup 1.0000x reference)
"""Trainium2 Bass kernel for 2-layer GCN (GCNConv -> relu -> GCNConv -> Linear).

v2 strategy (8 NeuronCores, SPMD):
  - Nodes padded to NPAD=100352, dealt serpentine-by-degree into 784 blocks of
    128 slots; 98 blocks per core (edge partition by destination).
  - Layer 1 aggregates RAW features: table xd = (x * dinv) bf16 in node order,
    gathered per edge with dma_gather (4 node-id quarter subtables, int16 idx,
    4 SWDGE queues).  Selection-matrix matmuls accumulate the TRANSPOSED
    aggregate in PSUM (lhsT = gathered chunk, rhs = is_equal mask), so W1 can
    be applied directly: p1T = W1^T @ aggT.  b1 enters as a rank-1 matmul lane
    (b1 x sqrt(deg)); relu runs on the Act engine (relu homogeneity defers the
    dst dinv); u2 = relu(...)^T @ W2f * dinv^2 where W2f = W2 @ Wfc.
  - One AllGather (Shared output) exchanges the u2 table, which is then
    copied to a plain Local DRAM tensor (gathers from collective-written
    pages measure ~2x slower); layer-2 subtables are core-QUAD slices of the
    paired view (25088 rows, int16-addressable).  Self-loops for layer 2 use
    an identity-matmul chunk on the SBUF-resident u2 panel.
  - Layer 2 aggregates untransposed (lhsT = mask, rhs = chunk[:, :64]); the
    b2f = b2 @ Wfc + bfc bias enters as a rank-1 lane; y = agg * dinv.
  - Host un-permutes rows.
"""

import os as _env_os
import numpy as np
import ml_dtypes

P = 128
NCORES = 8
NQ = 4
IN_C, HID, OUT_C = 128, 128, 64
CALL = int(_env_os.environ.get("KB_CALL", "1024"))   # rows per dma_gather call


def _set_size(n_nodes, bpc):
    global N, BPC, NBINS, NPAD, SHARD, QROWS
    N = n_nodes
    BPC = bpc
    NBINS = NCORES * BPC
    NPAD = NBINS * P
    SHARD = BPC * P
    QROWS = NPAD // NQ
    assert N <= NPAD and QROWS <= 32768 and 2 * SHARD <= 32768


_set_size(100000, 98)

_kernel_cache = {}


def _wrap_idx(st):
    """[C, NQ, SLEN] int16 -> [C, NQ, 128, SLEN//16] wrapped+replicated."""
    C, Q, SLEN = st.shape
    w = st.reshape(C, Q, SLEN // 16, 16)
    w = np.swapaxes(w, 2, 3)                       # [C, Q, 16, SLEN//16]
    return np.ascontiguousarray(np.tile(w, (1, 1, 8, 1)))


def _edge_arrays(q, lidx, core, b, dslot):
    """Per-core gather-index streams and dst-slot arrays for one layer.

    Streams are grouped by (core, quarter, block); each (block, quarter) cell
    is padded to K*128 lanes (dummy idx 0, dst-slot 255 -> zero mask row)."""
    lidx = lidx.astype(np.int16)
    cell = (core * NQ + q) * BPC + b
    ncell = NCORES * NQ * BPC
    counts = np.bincount(cell, minlength=ncell)
    K = int(np.ceil(counts.max() / P))
    CAP = K * P
    order = np.argsort(cell, kind="stable")
    start = np.zeros(ncell + 1, np.int64)
    np.cumsum(counts, out=start[1:])
    rank = np.arange(cell.shape[0]) - start[cell[order]]
    pos = cell[order] * CAP + rank
    idx_arr = np.zeros(ncell * CAP, np.int16)
    dl_arr = np.full(ncell * CAP, 255.0, np.float32)
    idx_arr[pos] = lidx[order]
    dl_arr[pos] = dslot[order]
    # pad each (core, quarter) stream to a multiple of CALL (uniform calls)
    slen = BPC * CAP
    slen_pad = -(-slen // CALL) * CALL
    st = np.zeros((NCORES, NQ, slen_pad), np.int16)
    st[:, :, :slen] = idx_arr.reshape(NCORES, NQ, slen)
    gidx = _wrap_idx(st)
    dl = dl_arr.reshape(NCORES, NQ, BPC, K, P)
    dl = dl.transpose(0, 4, 2, 1, 3).reshape(NCORES, P, BPC * NQ * K)
    return K, gidx, dl.astype(ml_dtypes.bfloat16)


def _preprocess(x, edge_index, W1, b1, W2, b2, Wfc, bfc):
    src = np.asarray(edge_index[0], dtype=np.int64)
    dst = np.asarray(edge_index[1], dtype=np.int64)
    deg = (np.bincount(dst, minlength=N) + 1).astype(np.float32)
    dinv_pad = np.ones(NPAD, np.float32)
    dinv_pad[:N] = (1.0 / np.sqrt(deg)).astype(np.float32)

    loop = np.arange(N, dtype=np.int64)
    src_a = np.concatenate([src, loop])
    dst_a = np.concatenate([dst, loop])

    # serpentine deal by degree -> (bin, slot); balances per-block edge counts
    key = np.zeros(NPAD, np.float32)
    key[:N] = deg
    order = np.argsort(-key, kind="stable")
    i = np.arange(NPAD)
    r, c = i // NBINS, i % NBINS
    bins_for_rank = np.where(r % 2 == 0, c, NBINS - 1 - c)
    perm_bin = np.empty(NPAD, np.int64)
    perm_slot = np.empty(NPAD, np.int64)
    perm_bin[order] = bins_for_rank
    perm_slot[order] = r
    perm_pos = perm_bin * P + perm_slot
    pos2node = np.empty(NPAD, np.int64)
    pos2node[perm_pos] = np.arange(NPAD)

    # layer 1 (self-loops included): subtables = node-id quarters of xd
    ecore_a = perm_bin[dst_a] // BPC
    eb_a = perm_bin[dst_a] % BPC
    edslot_a = perm_slot[dst_a].astype(np.float32)
    K1, gidx1, dl1 = _edge_arrays(src_a // QROWS, src_a % QROWS,
                                  ecore_a, eb_a, edslot_a)

    # layer 2 (NO self-loops; they come from the resident u2 panel):
    # the 64-wide AllGather output is viewed as [8*SHARD/2, 128] so one 256B
    # gather row holds a PAIR of nodes; streams are (src-quad, parity)-pure
    # and the matmul picks the correct 64-column half.
    ecore_e = perm_bin[dst] // BPC
    eb_e = perm_bin[dst] % BPC
    edslot_e = perm_slot[dst].astype(np.float32)
    spos = perm_pos[src]
    ppos = spos // 2
    q4 = (ppos // (2 * SHARD)) * 2 + (spos % 2)      # stream: quad*2 + parity
    K2, gidx2, dl2 = _edge_arrays(q4, ppos % (2 * SHARD),
                                  ecore_e, eb_e, edslot_e)

    # gather table: xd = x * dinv (bf16, node order)
    xd = np.zeros((NPAD, IN_C), np.float32)
    xd[:N] = np.asarray(x, np.float32)
    xd *= dinv_pad[:, None]
    xd = xd.astype(ml_dtypes.bfloat16)

    dinv_pos = dinv_pad[pos2node]                            # dinv by position
    invd_pos = 1.0 / dinv_pos                                # sqrt(deg)
    dinv_blk = dinv_pos.reshape(NCORES, BPC, P).transpose(0, 2, 1)  # [C,128,98]

    W2f = (np.asarray(W2, np.float32) @ np.asarray(Wfc, np.float32))
    b2f = np.asarray(b2, np.float32) @ np.asarray(Wfc, np.float32) \
        + np.asarray(bfc, np.float32)

    common = {
        "W1": np.asarray(W1, np.float32).astype(ml_dtypes.bfloat16),
        "W2f": W2f.astype(ml_dtypes.bfloat16),
        "b1r": np.asarray(b1, np.float32).reshape(1, HID).astype(ml_dtypes.bfloat16),
        "b2fr": b2f.reshape(1, OUT_C).astype(ml_dtypes.bfloat16),
        "iota": np.tile(np.arange(P, dtype=np.float32)[None, :], (P, 1)).astype(
            ml_dtypes.bfloat16
        ),
        "identb": np.eye(P, dtype=np.float32).astype(ml_dtypes.bfloat16),
        "xd": xd,
    }
    in_maps = []
    for c in range(NCORES):
        m = dict(common)
        m["dinvb"] = np.ascontiguousarray(dinv_blk[c])
        m["dinv2b"] = np.ascontiguousarray(dinv_blk[c] ** 2)
        m["invdr"] = np.ascontiguousarray(
            invd_pos.reshape(NCORES, SHARD)[c].reshape(1, SHARD)).astype(
            ml_dtypes.bfloat16)
        m["dstloc1"] = np.ascontiguousarray(dl1[c])
        m["dstloc2"] = np.ascontiguousarray(dl2[c])
        m["gidx1"] = np.ascontiguousarray(gidx1[c])
        m["gidx2"] = np.ascontiguousarray(gidx2[c])
        in_maps.append(m)
    return K1, K2, in_maps, perm_pos


def _build(K1, K2):
    import os
    import concourse.bass as bass  # noqa: F401
    import concourse.mybir as mybir
    import concourse.tile as tile
    from concourse import bacc

    stop_after = os.environ.get("KB_STOP_AFTER", "")   # "", "B", "CC"
    deep = os.environ.get("KB_DEEP", "0") == "1"       # deeper psum/mask pipelines
    sim1 = os.environ.get("KB_SIM", "") == "1"         # single-core TimelineSim
    nqueues = int(os.environ.get("KB_QUEUES", "4"))
    dt = mybir.dt
    OP = mybir.AluOpType
    AF = mybir.ActivationFunctionType
    _pad = lambda s: -(-s // CALL) * CALL
    SLEN1, SLEN2 = _pad(BPC * K1 * P), _pad(BPC * K2 * P)
    CPP = CALL // P     # chunks per gather call

    nc = bacc.Bacc("TRN2", num_devices=1 if sim1 else NCORES,
                   target_bir_lowering=False, debug=False,
                   num_swdge_queues=nqueues,
                   dynamic_dma_scratch_size=int(
                       os.environ.get("KB_SCRATCH", "16384")))

    xd = nc.dram_tensor("xd", [NPAD, IN_C], dt.bfloat16, kind="ExternalInput")
    W1 = nc.dram_tensor("W1", [IN_C, HID], dt.bfloat16, kind="ExternalInput")
    W2f = nc.dram_tensor("W2f", [HID, OUT_C], dt.bfloat16, kind="ExternalInput")
    b1r = nc.dram_tensor("b1r", [1, HID], dt.bfloat16, kind="ExternalInput")
    b2fr = nc.dram_tensor("b2fr", [1, OUT_C], dt.bfloat16, kind="ExternalInput")
    iota = nc.dram_tensor("iota", [P, P], dt.bfloat16, kind="ExternalInput")
    identb = nc.dram_tensor("identb", [P, P], dt.bfloat16, kind="ExternalInput")
    dinvb = nc.dram_tensor("dinvb", [P, BPC], dt.float32, kind="ExternalInput")
    dinv2b = nc.dram_tensor("dinv2b", [P, BPC], dt.float32, kind="ExternalInput")
    invdr = nc.dram_tensor("invdr", [1, SHARD], dt.bfloat16, kind="ExternalInput")
    dstloc1 = nc.dram_tensor("dstloc1", [P, BPC * NQ * K1], dt.bfloat16, kind="ExternalInput")
    dstloc2 = nc.dram_tensor("dstloc2", [P, BPC * NQ * K2], dt.bfloat16, kind="ExternalInput")
    gidx1 = nc.dram_tensor("gidx1", [NQ, P, SLEN1 // 16], dt.int16, kind="ExternalInput")
    gidx2 = nc.dram_tensor("gidx2", [NQ, P, SLEN2 // 16], dt.int16, kind="ExternalInput")
    y = nc.dram_tensor("y", [SHARD, OUT_C], dt.float32, kind="ExternalOutput")

    u2loc = nc.dram_tensor("u2loc", [SHARD, OUT_C], dt.bfloat16)
    shared_out = (not sim1) and os.environ.get("KB_SHARED", "1") == "1"
    t2copy = os.environ.get("KB_T2COPY", "1") == "1"
    T2p = nc.dram_tensor("T2p", [NCORES * SHARD // 2, P], dt.bfloat16,
                         addr_space="Shared" if shared_out else "Local")
    T2l = (nc.dram_tensor("T2l", [NCORES * SHARD // 2, P], dt.bfloat16)
           if t2copy else T2p)

    def agg_pass(sb_g, sb_s, ps, iota_t, tables, Kc, gidx_t, dstloc_ap, elem,
                 mm_block, nblocks=BPC):
        """For each block: gather message chunks (dma_gather per 1024 rows per
        quarter), build the selection mask, then run mm_block(b, s_all,
        chunk_fn) which issues the matmuls + epilogue."""
        slen = _pad(BPC * Kc * P)
        ncalls = slen // CALL
        gtiles = {}
        issued = [0] * NQ

        def issue(q, call):
            n = min(CALL, slen - call * CALL)
            gt = sb_g.tile([P, CPP, elem], dt.bfloat16, tag="gbuf")
            nc.gpsimd.dma_gather(
                out_ap=gt[:, : n // P, :],
                in_ap=tables[q],
                idxs_ap=gidx_t[q][:, call * (CALL // 16) : call * (CALL // 16) + n // 16],
                num_idxs=n,
                num_idxs_reg=n,
                elem_size=elem,
                queue_num=q % nqueues,
            )
            gtiles[(q, call)] = gt

        for b in range(nblocks):
            last_call = ((b + 1) * Kc - 1) // CPP
            for q in range(NQ):
                while issued[q] <= last_call and issued[q] < ncalls:
                    issue(q, issued[q])
                    issued[q] += 1
            s_all = sb_s.tile([P, NQ * Kc, P], dt.bfloat16, tag="sall")
            nc.vector.tensor_tensor(
                out=s_all[:],
                in0=dstloc_ap[:, b * NQ * Kc : (b + 1) * NQ * Kc].to_broadcast(
                    [P, NQ * Kc, P]
                ),
                in1=iota_t[:].rearrange("p (a b) -> p a b", a=1).to_broadcast(
                    [P, NQ * Kc, P]
                ),
                op=OP.is_equal,
            )

            def chunk(q, j):
                g = b * Kc + j
                return gtiles[(q, g // CPP)][:, g % CPP, :]

            mm_block(b, s_all, chunk)

    with tile.TileContext(nc) as tc:
        with tc.tile_pool(name="const", bufs=1) as cp:
            gconst = {}
            for name, t, shape, dtt in [
                ("W1", W1, [IN_C, HID], dt.bfloat16),
                ("W2f", W2f, [HID, OUT_C], dt.bfloat16),
                ("b1r", b1r, [1, HID], dt.bfloat16),
                ("b2fr", b2fr, [1, OUT_C], dt.bfloat16),
                ("iota", iota, [P, P], dt.bfloat16),
                ("identb", identb, [P, P], dt.bfloat16),
                ("dinvb", dinvb, [P, BPC], dt.float32),
                ("dinv2b", dinv2b, [P, BPC], dt.float32),
                ("invdr", invdr, [1, SHARD], dt.bfloat16),
                ("dstloc1", dstloc1, [P, BPC * NQ * K1], dt.bfloat16),
                ("dstloc2", dstloc2, [P, BPC * NQ * K2], dt.bfloat16),
            ]:
                tl = cp.tile(shape, dtt, tag=name)
                nc.sync.dma_start(out=tl[:], in_=t[:])
                gconst[name] = tl
            for _rep in range(int(os.environ.get("KB_REPEAT", "1"))):
                # ------------- phase B: layer-1 aggregation + u2 table -------------
                with (
                  tc.tile_pool(name="u2p", bufs=1) as u2pool,
                  tc.tile_pool(name="gx2", bufs=1) as gx2,
                ):
                  u2panel = u2pool.tile([P, BPC * OUT_C], dt.bfloat16, tag="u2panel")
                  gidx2_t = []
                  for q in range(NQ):
                      tl = gx2.tile([P, SLEN2 // 16], dt.int16, tag=f"gidx2_{q}")
                      nc.sync.dma_start(out=tl[:], in_=gidx2[q])
                      gidx2_t.append(tl)
                  with (
                    tc.tile_pool(name="phB", bufs=24 * 1024 // CALL) as pB,
                    tc.tile_pool(name="phBs", bufs=4 if deep else 3) as pBs,
                    tc.tile_pool(name="phBe", bufs=4) as pBe,
                    tc.tile_pool(name="gx1", bufs=1) as gx1,
                    tc.tile_pool(name="psB", bufs=2, space="PSUM") as psB,
                    tc.tile_pool(name="psBa", bufs=4 if deep else 3, space="PSUM") as psBa,
                  ):
                    gidx1_t = []
                    for q in range(NQ):
                        tl = gx1.tile([P, SLEN1 // 16], dt.int16, tag=f"gidx1_{q}")
                        nc.sync.dma_start(out=tl[:], in_=gidx1[q])
                        gidx1_t.append(tl)
                    tc.strict_bb_all_engine_barrier()

                    def mm1(b, s_all, chunk):
                        # transposed aggregate: aggT[ch, slot] = sum xd[src]
                        psum_aggT = psBa.tile([P, P], dt.float32, space="PSUM", tag="agg")
                        nmm = NQ * K1
                        k = 0
                        for q in range(NQ):
                            for j in range(K1):
                                nc.tensor.matmul(
                                    out=psum_aggT[:],
                                    lhsT=chunk(q, j),
                                    rhs=s_all[:, q * K1 + j, :],
                                    start=(k == 0),
                                    stop=(k == nmm - 1),
                                )
                                k += 1
                        aT_s = pBe.tile([P, P], dt.bfloat16, tag="aTs")
                        nc.vector.tensor_copy(out=aT_s[:], in_=psum_aggT[:])
                        # p1T[h, slot] = W1^T @ aggT + b1 x sqrt(deg)
                        p1T = psB.tile([P, P], dt.float32, space="PSUM", tag="p1T")
                        nc.tensor.matmul(
                            out=p1T[:], lhsT=gconst["W1"][:], rhs=aT_s[:],
                            start=True, stop=False,
                        )
                        nc.tensor.matmul(
                            out=p1T[:], lhsT=gconst["b1r"][:],
                            rhs=gconst["invdr"][:, b * P : (b + 1) * P],
                            start=False, stop=True,
                        )
                        t1T = pBe.tile([P, P], dt.bfloat16, tag="t1T")
                        nc.scalar.activation(out=t1T[:], in_=p1T[:], func=AF.Relu)
                        # u2[slot, :] = dinv^2 * (relu^T @ W2f)
                        pu = psB.tile([P, OUT_C], dt.float32, space="PSUM", tag="pu")
                        nc.tensor.matmul(
                            out=pu[:], lhsT=t1T[:], rhs=gconst["W2f"][:],
                            start=True, stop=True,
                        )
                        nc.vector.tensor_scalar(
                            out=u2panel[:, b * OUT_C : (b + 1) * OUT_C],
                            in0=pu[:],
                            scalar1=gconst["dinv2b"][:, b : b + 1],
                            scalar2=None,
                            op0=OP.mult,
                        )

                    agg_pass(pB, pBs, psBa, gconst["iota"],
                             [xd[q * QROWS : (q + 1) * QROWS, :] for q in range(NQ)],
                             K1, gidx1_t, gconst["dstloc1"][:], HID, mm1,
                             nblocks=int(os.environ.get("KB_B_BLOCKS", BPC)))
                    nc.sync.dma_start(
                        out=u2loc.ap().rearrange("(b p) h -> p b h", p=P),
                        in_=u2panel[:].rearrange("p (b h) -> p b h", h=OUT_C),
                    )

                  tc.strict_bb_all_engine_barrier()
                  run_cc = stop_after != "B"
                  if not run_cc:
                    with tc.tile_pool(name="dbgB", bufs=1) as dbg:
                        z = dbg.tile([P, BPC * OUT_C], dt.float32, tag="zB")
                        nc.vector.memset(z[:], 0)
                        nc.sync.dma_start(
                            out=y.ap().rearrange("(b p) h -> p b h", p=P),
                            in_=z[:].rearrange("p (b h) -> p b h", h=OUT_C),
                        )
                  if run_cc:
                    if sim1:
                        for c in range(NCORES):
                            nc.sync.dma_start(
                                out=T2p[c * SHARD // 2 : (c + 1) * SHARD // 2, :],
                                in_=u2loc.ap().rearrange(
                                    "(r two) h -> r (two h)", two=2),
                            )
                    else:
                        nc.gpsimd.collective_compute(
                            "AllGather",
                            mybir.AluOpType.bypass,
                            replica_groups=[list(range(NCORES))],
                            ins=[u2loc[:]],
                            outs=[T2p.ap().rearrange("r (two h) -> (r two) h", two=2)],
                        )
                  if t2copy and run_cc:
                      CH = NCORES * SHARD // 2 // 4
                      for c in range(4):
                          nc.sync.dma_start(
                              out=T2l[c * CH : (c + 1) * CH, :],
                              in_=T2p[c * CH : (c + 1) * CH, :],
                          )
                  if os.environ.get("KB_NOBAR", "1") != "1":
                      tc.strict_bb_all_engine_barrier()
                  run_d = stop_after not in ("B", "CC")
                  if run_cc and not run_d:
                    with tc.tile_pool(name="dbgC", bufs=1) as dbg:
                        z = dbg.tile([P, BPC * OUT_C], dt.float32, tag="zC")
                        nc.vector.memset(z[:], 0)
                        nc.sync.dma_start(
                            out=y.ap().rearrange("(b p) h -> p b h", p=P),
                            in_=z[:].rearrange("p (b h) -> p b h", h=OUT_C),
                        )

                  # ------------- phase D: layer-2 aggregation + y -------------
                  if run_d:
                    with (
                        tc.tile_pool(name="phD", bufs=24 * 1024 // CALL) as pD,
                        tc.tile_pool(name="phDs", bufs=5 if deep else 3) as pDs,
                        tc.tile_pool(name="psDa", bufs=6 if deep else 3, space="PSUM") as psDa,
                        tc.tile_pool(name="ypl", bufs=1) as ypool,
                    ):
                            ypanel = ypool.tile([P, BPC * OUT_C], dt.float32, tag="ypanel")

                            def mm2(b, s_all, chunk):
                                agg2 = psDa.tile([P, OUT_C], dt.float32, space="PSUM", tag="agg2")
                                nmm = NQ * K2
                                k = 0
                                for q in range(NQ):
                                    for j in range(K2):
                                        nc.tensor.matmul(
                                            out=agg2[:],
                                            lhsT=s_all[:, q * K2 + j, :],
                                            rhs=chunk(q, j)[
                                                :, (q % 2) * OUT_C
                                                : (q % 2 + 1) * OUT_C],
                                            start=(k == 0),
                                            stop=False,
                                        )
                                        k += 1
                                # self-loop from the resident u2 panel
                                nc.tensor.matmul(
                                    out=agg2[:],
                                    lhsT=gconst["identb"][:],
                                    rhs=u2panel[:, b * OUT_C : (b + 1) * OUT_C],
                                    start=False,
                                    stop=False,
                                )
                                # + sqrt(deg) x b2f  (-> +b2f after the dinv scale)
                                nc.tensor.matmul(
                                    out=agg2[:],
                                    lhsT=gconst["invdr"][:, b * P : (b + 1) * P],
                                    rhs=gconst["b2fr"][:],
                                    start=False,
                                    stop=True,
                                )
                                nc.vector.tensor_scalar(
                                    out=ypanel[:, b * OUT_C : (b + 1) * OUT_C],
                                    in0=agg2[:],
                                    scalar1=gconst["dinvb"][:, b : b + 1],
                                    scalar2=None,
                                    op0=OP.mult,
                                )

                            agg_pass(pD, pDs, psDa, gconst["iota"],
                                     [T2l[(qs // 2) * 2 * SHARD
                                          : (qs // 2 + 1) * 2 * SHARD, :]
                                      for qs in range(NQ)],
                                     K2, gidx2_t, gconst["dstloc2"][:], P, mm2,
                                     nblocks=int(os.environ.get("KB_D_BLOCKS", BPC)))
                            nc.sync.dma_start(
                                out=y.ap().rearrange("(b p) h -> p b h", p=P),
                                in_=ypanel[:].rearrange("p (b h) -> p b h", h=OUT_C),
                            )
                if os.environ.get("KB_REPBAR", "1") == "1":
                    tc.strict_bb_all_engine_barrier()

    nc.compile()
    return nc


def _make_runner(nc):
    """jit-compiled SPMD runner over 8 cores."""
    import jax
    import numpy as np
    from jax.sharding import Mesh, PartitionSpec
    from jax.experimental.shard_map import shard_map
    import concourse.mybir as mybir
    from concourse import bass2jax

    bass2jax.install_neuronx_cc_hook()
    partition_name = nc.partition_id_tensor.name if nc.partition_id_tensor else None
    in_names, out_names, out_avals, zero_outs = [], [], [], []
    for alloc in nc.m.functions[0].allocations:
        if not isinstance(alloc, mybir.MemoryLocationSet):
            continue
        name = alloc.memorylocations[0].name
        if alloc.kind == "ExternalInput":
            if name != partition_name:
                in_names.append(name)
        elif alloc.kind == "ExternalOutput":
            out_names.append(name)
            shape = tuple(alloc.tensor_shape)
            dtype = mybir.dt.np(alloc.dtype)
            out_avals.append(jax.core.ShapedArray(shape, dtype))
            zero_outs.append(np.zeros(shape, dtype))
    n_params = len(in_names)
    all_in_names = list(in_names) + list(out_names)
    if partition_name is not None:
        all_in_names.append(partition_name)

    def _body(*args):
        operands = list(args)
        if partition_name is not None:
            operands.append(bass2jax.partition_id_tensor())
        outs = bass2jax._bass_exec_p.bind(
            *operands,
            out_avals=tuple(out_avals),
            in_names=tuple(all_in_names),
            out_names=tuple(out_names),
            lowering_input_output_aliases=(),
            sim_require_finite=True,
            sim_require_nnan=True,
            nc=nc,
        )
        return tuple(outs)

    devices = jax.devices()[:NCORES]
    mesh = Mesh(np.asarray(devices), ("core",))
    in_specs = (PartitionSpec("core"),) * (n_params + len(out_names))
    out_specs = (PartitionSpec("core"),) * len(out_names)
    fn = jax.jit(
        shard_map(_body, mesh=mesh, in_specs=in_specs, out_specs=out_specs,
                  check_rep=False),
        keep_unused=True,
    )
    return fn, in_names, out_names, zero_outs, mesh


def kernel(x, edge_index, W1, b1, W2, b2, Wfc, bfc, _trace=False, _bench=True):
    import time as _time
    import jax
    from jax.sharding import NamedSharding, PartitionSpec

    import os as _os
    K1, K2, in_maps, perm_pos = _preprocess(x, edge_index, W1, b1, W2, b2, Wfc, bfc)
    key = (K1, K2, _os.environ.get("KB_REPEAT", "1"),
           _os.environ.get("KB_STOP_AFTER", ""), _os.environ.get("KB_D_BLOCKS", ""),
           _os.environ.get("KB_B_BLOCKS", ""), _os.environ.get("KB_QUEUES", "4"),
           _os.environ.get("KB_NOBAR", "1"), _os.environ.get("KB_REPBAR", "1"),
           _os.environ.get("KB_SCRATCH", ""), _os.environ.get("KB_SHARED", "1"),
           _os.environ.get("KB_T2COPY", "1"), _os.environ.get("KB_DEEP", "0"),
           CALL)
    if key not in _kernel_cache:
        nc = _build(K1, K2)
        _kernel_cache[key] = (nc, _make_runner(nc))
    nc, (fn, in_names, out_names, zero_outs, mesh) = _kernel_cache[key]

    sh = NamedSharding(mesh, PartitionSpec("core"))
    concat_in = [
        np.concatenate([np.asarray(in_maps[c][nm]) for c in range(NCORES)], axis=0)
        for nm in in_names
    ]
    concat_zeros = [
        np.zeros((NCORES * z.shape[0], *z.shape[1:]), z.dtype) for z in zero_outs
    ]
    dev_in = [jax.device_put(a, sh) for a in concat_in + concat_zeros]
    out_arrs = fn(*dev_in)
    jax.block_until_ready(out_arrs)

    if _bench:
        times = []
        for _ in range(5):
            t0 = _time.perf_counter()
            out_arrs = fn(*dev_in)
            jax.block_until_ready(out_arrs)
            times.append(_time.perf_counter() - t0)
        kernel._last_times = times
        kernel._last_exec_time_ns = int(min(times) * 1e9)
    else:
        kernel._last_exec_time_ns = None
    if not hasattr(kernel, "_runners"):
        kernel._runners = {}
    kernel._runners[_os.environ.get("KB_REPEAT", "1")] = (fn, dev_in)

    outs = {nm: np.asarray(out_arrs[i]) for i, nm in enumerate(out_names)}
    Y = outs["y"].reshape(NCORES, SHARD, OUT_C).reshape(NCORES * SHARD, OUT_C)
    return Y[perm_pos[:N]].astype(np.float32)


# revision 14
# speedup vs baseline: 1.3125x; 1.3125x over previous
"""Trainium2 Bass kernel for 2-layer GCN (GCNConv -> relu -> GCNConv -> Linear).

v2 strategy (8 NeuronCores, SPMD):
  - Nodes padded to NPAD=100352, dealt serpentine-by-degree into 784 blocks of
    128 slots; 98 blocks per core (edge partition by destination).
  - Layer 1 aggregates RAW features: table xd = (x * dinv) bf16 in node order,
    gathered per edge with dma_gather (4 node-id quarter subtables, int16 idx,
    4 SWDGE queues).  Selection-matrix matmuls accumulate the TRANSPOSED
    aggregate in PSUM (lhsT = gathered chunk, rhs = is_equal mask), so W1 can
    be applied directly: p1T = W1^T @ aggT.  b1 enters as a rank-1 matmul lane
    (b1 x sqrt(deg)); relu runs on the Act engine (relu homogeneity defers the
    dst dinv); u2 = relu(...)^T @ W2f * dinv^2 where W2f = W2 @ Wfc.
  - One AllGather (Shared output) exchanges the u2 table, which is then
    copied to a plain Local DRAM tensor (gathers from collective-written
    pages measure ~2x slower); layer-2 subtables are core-QUAD slices of the
    paired view (25088 rows, int16-addressable).  Self-loops for layer 2 use
    an identity-matmul chunk on the SBUF-resident u2 panel.
  - Layer 2 aggregates untransposed (lhsT = mask, rhs = chunk[:, :64]); the
    b2f = b2 @ Wfc + bfc bias enters as a rank-1 lane; y = agg * dinv.
  - Host un-permutes rows.
"""

import os as _env_os
import numpy as np
import ml_dtypes

P = 128
NCORES = 8
NQ = 4
IN_C, HID, OUT_C = 128, 128, 64
CALL = int(_env_os.environ.get("KB_CALL", "1024"))   # rows per dma_gather call


def _set_size(n_nodes, bpc):
    global N, BPC, NBINS, NPAD, SHARD, QROWS
    N = n_nodes
    BPC = bpc
    NBINS = NCORES * BPC
    NPAD = NBINS * P
    SHARD = BPC * P
    QROWS = NPAD // NQ
    assert N <= NPAD and QROWS <= 32768 and 2 * SHARD <= 32768


_set_size(100000, 98)

_kernel_cache = {}


def _wrap_idx(st):
    """[C, NQ, SLEN] int16 -> [C, NQ, 128, SLEN//16] wrapped+replicated."""
    C, Q, SLEN = st.shape
    w = st.reshape(C, Q, SLEN // 16, 16)
    w = np.swapaxes(w, 2, 3)                       # [C, Q, 16, SLEN//16]
    return np.ascontiguousarray(np.tile(w, (1, 1, 8, 1)))


def _edge_arrays(q, lidx, core, b, dslot):
    """Per-core gather-index streams and dst-slot arrays for one layer.

    Streams are grouped by (core, quarter, block); each (block, quarter) cell
    is padded to K*128 lanes (dummy idx 0, dst-slot 255 -> zero mask row)."""
    lidx = lidx.astype(np.int16)
    cell = (core * NQ + q) * BPC + b
    ncell = NCORES * NQ * BPC
    counts = np.bincount(cell, minlength=ncell)
    K = int(np.ceil(counts.max() / P))
    CAP = K * P
    order = np.argsort(cell, kind="stable")
    start = np.zeros(ncell + 1, np.int64)
    np.cumsum(counts, out=start[1:])
    rank = np.arange(cell.shape[0]) - start[cell[order]]
    pos = cell[order] * CAP + rank
    idx_arr = np.zeros(ncell * CAP, np.int16)
    dl_arr = np.full(ncell * CAP, 255.0, np.float32)
    idx_arr[pos] = lidx[order]
    dl_arr[pos] = dslot[order]
    # pad each (core, quarter) stream to a multiple of CALL (uniform calls)
    slen = BPC * CAP
    slen_pad = -(-slen // CALL) * CALL
    st = np.zeros((NCORES, NQ, slen_pad), np.int16)
    st[:, :, :slen] = idx_arr.reshape(NCORES, NQ, slen)
    gidx = _wrap_idx(st)
    dl = dl_arr.reshape(NCORES, NQ, BPC, K, P)
    dl = dl.transpose(0, 4, 2, 1, 3).reshape(NCORES, P, BPC * NQ * K)
    return K, gidx, dl.astype(ml_dtypes.bfloat16)


def _preprocess(x, edge_index, W1, b1, W2, b2, Wfc, bfc):
    src = np.asarray(edge_index[0], dtype=np.int64)
    dst = np.asarray(edge_index[1], dtype=np.int64)
    deg = (np.bincount(dst, minlength=N) + 1).astype(np.float32)
    dinv_pad = np.ones(NPAD, np.float32)
    dinv_pad[:N] = (1.0 / np.sqrt(deg)).astype(np.float32)

    loop = np.arange(N, dtype=np.int64)
    src_a = np.concatenate([src, loop])
    dst_a = np.concatenate([dst, loop])

    # serpentine deal by degree -> (bin, slot); balances per-block edge counts
    key = np.zeros(NPAD, np.float32)
    key[:N] = deg
    order = np.argsort(-key, kind="stable")
    i = np.arange(NPAD)
    r, c = i // NBINS, i % NBINS
    bins_for_rank = np.where(r % 2 == 0, c, NBINS - 1 - c)
    perm_bin = np.empty(NPAD, np.int64)
    perm_slot = np.empty(NPAD, np.int64)
    perm_bin[order] = bins_for_rank
    perm_slot[order] = r
    perm_pos = perm_bin * P + perm_slot
    pos2node = np.empty(NPAD, np.int64)
    pos2node[perm_pos] = np.arange(NPAD)

    # layer 1 (self-loops included): subtables = node-id quarters of xd
    ecore_a = perm_bin[dst_a] // BPC
    eb_a = perm_bin[dst_a] % BPC
    edslot_a = perm_slot[dst_a].astype(np.float32)
    K1, gidx1, dl1 = _edge_arrays(src_a // QROWS, src_a % QROWS,
                                  ecore_a, eb_a, edslot_a)

    # layer 2 (NO self-loops; they come from the resident u2 panel):
    # the 64-wide AllGather output is viewed as [8*SHARD/2, 128] so one 256B
    # gather row holds a PAIR of nodes; streams are (src-quad, parity)-pure
    # and the matmul picks the correct 64-column half.
    ecore_e = perm_bin[dst] // BPC
    eb_e = perm_bin[dst] % BPC
    edslot_e = perm_slot[dst].astype(np.float32)
    spos = perm_pos[src]
    ppos = spos // 2
    q4 = (ppos // (2 * SHARD)) * 2 + (spos % 2)      # stream: quad*2 + parity
    K2, gidx2, dl2 = _edge_arrays(q4, ppos % (2 * SHARD),
                                  ecore_e, eb_e, edslot_e)

    # gather table: xd = x * dinv (bf16, node order)
    xd = np.zeros((NPAD, IN_C), np.float32)
    xd[:N] = np.asarray(x, np.float32)
    xd *= dinv_pad[:, None]
    xd = xd.astype(ml_dtypes.bfloat16)

    dinv_pos = dinv_pad[pos2node]                            # dinv by position
    invd_pos = 1.0 / dinv_pos                                # sqrt(deg)
    dinv_blk = dinv_pos.reshape(NCORES, BPC, P).transpose(0, 2, 1)  # [C,128,98]

    W2f = (np.asarray(W2, np.float32) @ np.asarray(Wfc, np.float32))
    b2f = np.asarray(b2, np.float32) @ np.asarray(Wfc, np.float32) \
        + np.asarray(bfc, np.float32)

    common = {
        "W1": np.asarray(W1, np.float32).astype(ml_dtypes.bfloat16),
        "W2f": W2f.astype(ml_dtypes.bfloat16),
        "b1r": np.asarray(b1, np.float32).reshape(1, HID).astype(ml_dtypes.bfloat16),
        "b2fr": b2f.reshape(1, OUT_C).astype(ml_dtypes.bfloat16),
        "iota": np.tile(np.arange(P, dtype=np.float32)[None, :], (P, 1)).astype(
            ml_dtypes.bfloat16
        ),
        "identb": np.eye(P, dtype=np.float32).astype(ml_dtypes.bfloat16),
        "xd": xd,
    }
    in_maps = []
    for c in range(NCORES):
        m = dict(common)
        m["dinvb"] = np.ascontiguousarray(dinv_blk[c])
        m["dinv2b"] = np.ascontiguousarray(dinv_blk[c] ** 2)
        m["invdr"] = np.ascontiguousarray(
            invd_pos.reshape(NCORES, SHARD)[c].reshape(1, SHARD)).astype(
            ml_dtypes.bfloat16)
        m["dstloc1"] = np.ascontiguousarray(dl1[c])
        m["dstloc2"] = np.ascontiguousarray(dl2[c])
        m["gidx1"] = np.ascontiguousarray(gidx1[c])
        m["gidx2"] = np.ascontiguousarray(gidx2[c])
        in_maps.append(m)
    return K1, K2, in_maps, perm_pos


def _build(K1, K2):
    import os
    import concourse.bass as bass  # noqa: F401
    import concourse.mybir as mybir
    import concourse.tile as tile
    from concourse import bacc

    stop_after = os.environ.get("KB_STOP_AFTER", "")   # "", "B", "CC"
    deep = os.environ.get("KB_DEEP", "0") == "1"       # deeper psum/mask pipelines
    sim1 = os.environ.get("KB_SIM", "") == "1"         # single-core TimelineSim
    nqueues = int(os.environ.get("KB_QUEUES", "4"))
    dt = mybir.dt
    OP = mybir.AluOpType
    AF = mybir.ActivationFunctionType
    _pad = lambda s: -(-s // CALL) * CALL
    SLEN1, SLEN2 = _pad(BPC * K1 * P), _pad(BPC * K2 * P)
    CPP = CALL // P     # chunks per gather call

    nc = bacc.Bacc("TRN2", num_devices=1 if sim1 else NCORES,
                   target_bir_lowering=False, debug=False,
                   num_swdge_queues=nqueues,
                   dynamic_dma_scratch_size=int(
                       os.environ.get("KB_SCRATCH", "16384")))

    xd = nc.dram_tensor("xd", [NPAD, IN_C], dt.bfloat16, kind="ExternalInput")
    W1 = nc.dram_tensor("W1", [IN_C, HID], dt.bfloat16, kind="ExternalInput")
    W2f = nc.dram_tensor("W2f", [HID, OUT_C], dt.bfloat16, kind="ExternalInput")
    b1r = nc.dram_tensor("b1r", [1, HID], dt.bfloat16, kind="ExternalInput")
    b2fr = nc.dram_tensor("b2fr", [1, OUT_C], dt.bfloat16, kind="ExternalInput")
    iota = nc.dram_tensor("iota", [P, P], dt.bfloat16, kind="ExternalInput")
    identb = nc.dram_tensor("identb", [P, P], dt.bfloat16, kind="ExternalInput")
    dinvb = nc.dram_tensor("dinvb", [P, BPC], dt.float32, kind="ExternalInput")
    dinv2b = nc.dram_tensor("dinv2b", [P, BPC], dt.float32, kind="ExternalInput")
    invdr = nc.dram_tensor("invdr", [1, SHARD], dt.bfloat16, kind="ExternalInput")
    dstloc1 = nc.dram_tensor("dstloc1", [P, BPC * NQ * K1], dt.bfloat16, kind="ExternalInput")
    dstloc2 = nc.dram_tensor("dstloc2", [P, BPC * NQ * K2], dt.bfloat16, kind="ExternalInput")
    gidx1 = nc.dram_tensor("gidx1", [NQ, P, SLEN1 // 16], dt.int16, kind="ExternalInput")
    gidx2 = nc.dram_tensor("gidx2", [NQ, P, SLEN2 // 16], dt.int16, kind="ExternalInput")
    y = nc.dram_tensor("y", [SHARD, OUT_C], dt.float32, kind="ExternalOutput")

    u2loc = nc.dram_tensor("u2loc", [SHARD, OUT_C], dt.bfloat16)
    shared_out = (not sim1) and os.environ.get("KB_SHARED", "1") == "1"
    t2copy = os.environ.get("KB_T2COPY", "1") == "1"
    T2p = nc.dram_tensor("T2p", [NCORES * SHARD // 2, P], dt.bfloat16,
                         addr_space="Shared" if shared_out else "Local")
    T2l = (nc.dram_tensor("T2l", [NCORES * SHARD // 2, P], dt.bfloat16)
           if t2copy else T2p)

    def agg_pass(sb_g, sb_s, ps, iota_t, tables, Kc, gidx_t, dstloc_ap, elem,
                 mm_block, nblocks=BPC):
        """For each block: gather message chunks (dma_gather per 1024 rows per
        quarter), build the selection mask, then run mm_block(b, s_all,
        chunk_fn) which issues the matmuls + epilogue."""
        slen = _pad(BPC * Kc * P)
        ncalls = slen // CALL
        gtiles = {}
        issued = [0] * NQ

        def issue(q, call):
            n = min(CALL, slen - call * CALL)
            gt = sb_g.tile([P, CPP, elem], dt.bfloat16, tag="gbuf")
            nc.gpsimd.dma_gather(
                out_ap=gt[:, : n // P, :],
                in_ap=tables[q],
                idxs_ap=gidx_t[q][:, call * (CALL // 16) : call * (CALL // 16) + n // 16],
                num_idxs=n,
                num_idxs_reg=n,
                elem_size=elem,
                queue_num=q % nqueues,
            )
            gtiles[(q, call)] = gt

        for b in range(nblocks):
            last_call = ((b + 1) * Kc - 1) // CPP
            for q in range(NQ):
                while issued[q] <= last_call and issued[q] < ncalls:
                    issue(q, issued[q])
                    issued[q] += 1
            s_all = sb_s.tile([P, NQ * Kc, P], dt.bfloat16, tag="sall")
            nc.vector.tensor_tensor(
                out=s_all[:],
                in0=dstloc_ap[:, b * NQ * Kc : (b + 1) * NQ * Kc].to_broadcast(
                    [P, NQ * Kc, P]
                ),
                in1=iota_t[:].rearrange("p (a b) -> p a b", a=1).to_broadcast(
                    [P, NQ * Kc, P]
                ),
                op=OP.is_equal,
            )

            def chunk(q, j):
                g = b * Kc + j
                return gtiles[(q, g // CPP)][:, g % CPP, :]

            mm_block(b, s_all, chunk)

    with tile.TileContext(nc) as tc:
        with tc.tile_pool(name="const", bufs=1) as cp:
            gconst = {}
            for name, t, shape, dtt in [
                ("W1", W1, [IN_C, HID], dt.bfloat16),
                ("W2f", W2f, [HID, OUT_C], dt.bfloat16),
                ("b1r", b1r, [1, HID], dt.bfloat16),
                ("b2fr", b2fr, [1, OUT_C], dt.bfloat16),
                ("iota", iota, [P, P], dt.bfloat16),
                ("identb", identb, [P, P], dt.bfloat16),
                ("dinvb", dinvb, [P, BPC], dt.float32),
                ("dinv2b", dinv2b, [P, BPC], dt.float32),
                ("invdr", invdr, [1, SHARD], dt.bfloat16),
                ("dstloc1", dstloc1, [P, BPC * NQ * K1], dt.bfloat16),
                ("dstloc2", dstloc2, [P, BPC * NQ * K2], dt.bfloat16),
            ]:
                tl = cp.tile(shape, dtt, tag=name)
                nc.sync.dma_start(out=tl[:], in_=t[:])
                gconst[name] = tl
            for _rep in range(int(os.environ.get("KB_REPEAT", "1"))):
                # ------------- phase B: layer-1 aggregation + u2 table -------------
                with (
                  tc.tile_pool(name="u2p", bufs=1) as u2pool,
                  tc.tile_pool(name="gx2", bufs=1) as gx2,
                ):
                  u2panel = u2pool.tile([P, BPC * OUT_C], dt.bfloat16, tag="u2panel")
                  gidx2_t = []
                  for q in range(NQ):
                      tl = gx2.tile([P, SLEN2 // 16], dt.int16, tag=f"gidx2_{q}")
                      nc.sync.dma_start(out=tl[:], in_=gidx2[q])
                      gidx2_t.append(tl)
                  with (
                    tc.tile_pool(name="phB", bufs=24 if CALL <= 1024 else 8) as pB,
                    tc.tile_pool(name="phBs", bufs=4 if deep else 3) as pBs,
                    tc.tile_pool(name="phBe", bufs=4) as pBe,
                    tc.tile_pool(name="gx1", bufs=1) as gx1,
                    tc.tile_pool(name="psB", bufs=2, space="PSUM") as psB,
                    tc.tile_pool(name="psBa", bufs=4 if deep else 3, space="PSUM") as psBa,
                  ):
                    gidx1_t = []
                    for q in range(NQ):
                        tl = gx1.tile([P, SLEN1 // 16], dt.int16, tag=f"gidx1_{q}")
                        nc.sync.dma_start(out=tl[:], in_=gidx1[q])
                        gidx1_t.append(tl)
                    tc.strict_bb_all_engine_barrier()

                    def mm1(b, s_all, chunk):
                        # transposed aggregate: aggT[ch, slot] = sum xd[src]
                        psum_aggT = psBa.tile([P, P], dt.float32, space="PSUM", tag="agg")
                        nmm = NQ * K1
                        k = 0
                        for q in range(NQ):
                            for j in range(K1):
                                nc.tensor.matmul(
                                    out=psum_aggT[:],
                                    lhsT=chunk(q, j),
                                    rhs=s_all[:, q * K1 + j, :],
                                    start=(k == 0),
                                    stop=(k == nmm - 1),
                                )
                                k += 1
                        aT_s = pBe.tile([P, P], dt.bfloat16, tag="aTs")
                        nc.vector.tensor_copy(out=aT_s[:], in_=psum_aggT[:])
                        # p1T[h, slot] = W1^T @ aggT + b1 x sqrt(deg)
                        p1T = psB.tile([P, P], dt.float32, space="PSUM", tag="p1T")
                        nc.tensor.matmul(
                            out=p1T[:], lhsT=gconst["W1"][:], rhs=aT_s[:],
                            start=True, stop=False,
                        )
                        nc.tensor.matmul(
                            out=p1T[:], lhsT=gconst["b1r"][:],
                            rhs=gconst["invdr"][:, b * P : (b + 1) * P],
                            start=False, stop=True,
                        )
                        t1T = pBe.tile([P, P], dt.bfloat16, tag="t1T")
                        nc.scalar.activation(out=t1T[:], in_=p1T[:], func=AF.Relu)
                        # u2[slot, :] = dinv^2 * (relu^T @ W2f)
                        pu = psB.tile([P, OUT_C], dt.float32, space="PSUM", tag="pu")
                        nc.tensor.matmul(
                            out=pu[:], lhsT=t1T[:], rhs=gconst["W2f"][:],
                            start=True, stop=True,
                        )
                        nc.vector.tensor_scalar(
                            out=u2panel[:, b * OUT_C : (b + 1) * OUT_C],
                            in0=pu[:],
                            scalar1=gconst["dinv2b"][:, b : b + 1],
                            scalar2=None,
                            op0=OP.mult,
                        )

                    agg_pass(pB, pBs, psBa, gconst["iota"],
                             [xd[q * QROWS : (q + 1) * QROWS, :] for q in range(NQ)],
                             K1, gidx1_t, gconst["dstloc1"][:], HID, mm1,
                             nblocks=int(os.environ.get("KB_B_BLOCKS", BPC)))
                    nc.sync.dma_start(
                        out=u2loc.ap().rearrange("(b p) h -> p b h", p=P),
                        in_=u2panel[:].rearrange("p (b h) -> p b h", h=OUT_C),
                    )

                  tc.strict_bb_all_engine_barrier()
                  run_cc = stop_after != "B"
                  if not run_cc:
                    with tc.tile_pool(name="dbgB", bufs=1) as dbg:
                        z = dbg.tile([P, BPC * OUT_C], dt.float32, tag="zB")
                        nc.vector.memset(z[:], 0)
                        nc.sync.dma_start(
                            out=y.ap().rearrange("(b p) h -> p b h", p=P),
                            in_=z[:].rearrange("p (b h) -> p b h", h=OUT_C),
                        )
                  if run_cc:
                    if sim1:
                        for c in range(NCORES):
                            nc.sync.dma_start(
                                out=T2p[c * SHARD // 2 : (c + 1) * SHARD // 2, :],
                                in_=u2loc.ap().rearrange(
                                    "(r two) h -> r (two h)", two=2),
                            )
                    else:
                        nc.gpsimd.collective_compute(
                            "AllGather",
                            mybir.AluOpType.bypass,
                            replica_groups=[list(range(NCORES))],
                            ins=[u2loc[:]],
                            outs=[T2p.ap().rearrange("r (two h) -> (r two) h", two=2)],
                        )
                  if t2copy and run_cc:
                      CH = NCORES * SHARD // 2 // 4
                      for c in range(4):
                          nc.sync.dma_start(
                              out=T2l[c * CH : (c + 1) * CH, :],
                              in_=T2p[c * CH : (c + 1) * CH, :],
                          )
                  if os.environ.get("KB_NOBAR", "1") != "1":
                      tc.strict_bb_all_engine_barrier()
                  run_d = stop_after not in ("B", "CC")
                  if run_cc and not run_d:
                    with tc.tile_pool(name="dbgC", bufs=1) as dbg:
                        z = dbg.tile([P, BPC * OUT_C], dt.float32, tag="zC")
                        nc.vector.memset(z[:], 0)
                        nc.sync.dma_start(
                            out=y.ap().rearrange("(b p) h -> p b h", p=P),
                            in_=z[:].rearrange("p (b h) -> p b h", h=OUT_C),
                        )

                  # ------------- phase D: layer-2 aggregation + y -------------
                  if run_d:
                    with (
                        tc.tile_pool(name="phD", bufs=24 if CALL <= 1024 else 8) as pD,
                        tc.tile_pool(name="phDs", bufs=5 if deep else 3) as pDs,
                        tc.tile_pool(name="psDa", bufs=6 if deep else 3, space="PSUM") as psDa,
                        tc.tile_pool(name="ypl", bufs=1) as ypool,
                    ):
                            ypanel = ypool.tile([P, BPC * OUT_C], dt.float32, tag="ypanel")

                            def mm2(b, s_all, chunk):
                                agg2 = psDa.tile([P, OUT_C], dt.float32, space="PSUM", tag="agg2")
                                nmm = NQ * K2
                                k = 0
                                for q in range(NQ):
                                    for j in range(K2):
                                        nc.tensor.matmul(
                                            out=agg2[:],
                                            lhsT=s_all[:, q * K2 + j, :],
                                            rhs=chunk(q, j)[
                                                :, (q % 2) * OUT_C
                                                : (q % 2 + 1) * OUT_C],
                                            start=(k == 0),
                                            stop=False,
                                        )
                                        k += 1
                                # self-loop from the resident u2 panel
                                nc.tensor.matmul(
                                    out=agg2[:],
                                    lhsT=gconst["identb"][:],
                                    rhs=u2panel[:, b * OUT_C : (b + 1) * OUT_C],
                                    start=False,
                                    stop=False,
                                )
                                # + sqrt(deg) x b2f  (-> +b2f after the dinv scale)
                                nc.tensor.matmul(
                                    out=agg2[:],
                                    lhsT=gconst["invdr"][:, b * P : (b + 1) * P],
                                    rhs=gconst["b2fr"][:],
                                    start=False,
                                    stop=True,
                                )
                                nc.vector.tensor_scalar(
                                    out=ypanel[:, b * OUT_C : (b + 1) * OUT_C],
                                    in0=agg2[:],
                                    scalar1=gconst["dinvb"][:, b : b + 1],
                                    scalar2=None,
                                    op0=OP.mult,
                                )

                            agg_pass(pD, pDs, psDa, gconst["iota"],
                                     [T2l[(qs // 2) * 2 * SHARD
                                          : (qs // 2 + 1) * 2 * SHARD, :]
                                      for qs in range(NQ)],
                                     K2, gidx2_t, gconst["dstloc2"][:], P, mm2,
                                     nblocks=int(os.environ.get("KB_D_BLOCKS", BPC)))
                            nc.sync.dma_start(
                                out=y.ap().rearrange("(b p) h -> p b h", p=P),
                                in_=ypanel[:].rearrange("p (b h) -> p b h", h=OUT_C),
                            )
                if os.environ.get("KB_REPBAR", "1") == "1":
                    tc.strict_bb_all_engine_barrier()

    nc.compile()
    return nc


def _make_runner(nc):
    """jit-compiled SPMD runner over 8 cores."""
    import jax
    import numpy as np
    from jax.sharding import Mesh, PartitionSpec
    from jax.experimental.shard_map import shard_map
    import concourse.mybir as mybir
    from concourse import bass2jax

    bass2jax.install_neuronx_cc_hook()
    partition_name = nc.partition_id_tensor.name if nc.partition_id_tensor else None
    in_names, out_names, out_avals, zero_outs = [], [], [], []
    for alloc in nc.m.functions[0].allocations:
        if not isinstance(alloc, mybir.MemoryLocationSet):
            continue
        name = alloc.memorylocations[0].name
        if alloc.kind == "ExternalInput":
            if name != partition_name:
                in_names.append(name)
        elif alloc.kind == "ExternalOutput":
            out_names.append(name)
            shape = tuple(alloc.tensor_shape)
            dtype = mybir.dt.np(alloc.dtype)
            out_avals.append(jax.core.ShapedArray(shape, dtype))
            zero_outs.append(np.zeros(shape, dtype))
    n_params = len(in_names)
    all_in_names = list(in_names) + list(out_names)
    if partition_name is not None:
        all_in_names.append(partition_name)

    def _body(*args):
        operands = list(args)
        if partition_name is not None:
            operands.append(bass2jax.partition_id_tensor())
        outs = bass2jax._bass_exec_p.bind(
            *operands,
            out_avals=tuple(out_avals),
            in_names=tuple(all_in_names),
            out_names=tuple(out_names),
            lowering_input_output_aliases=(),
            sim_require_finite=True,
            sim_require_nnan=True,
            nc=nc,
        )
        return tuple(outs)

    devices = jax.devices()[:NCORES]
    mesh = Mesh(np.asarray(devices), ("core",))
    in_specs = (PartitionSpec("core"),) * (n_params + len(out_names))
    out_specs = (PartitionSpec("core"),) * len(out_names)
    fn = jax.jit(
        shard_map(_body, mesh=mesh, in_specs=in_specs, out_specs=out_specs,
                  check_rep=False),
        keep_unused=True,
    )
    return fn, in_names, out_names, zero_outs, mesh


def kernel(x, edge_index, W1, b1, W2, b2, Wfc, bfc, _trace=False, _bench=True):
    import time as _time
    import jax
    from jax.sharding import NamedSharding, PartitionSpec

    import os as _os
    K1, K2, in_maps, perm_pos = _preprocess(x, edge_index, W1, b1, W2, b2, Wfc, bfc)
    key = (K1, K2, _os.environ.get("KB_REPEAT", "1"),
           _os.environ.get("KB_STOP_AFTER", ""), _os.environ.get("KB_D_BLOCKS", ""),
           _os.environ.get("KB_B_BLOCKS", ""), _os.environ.get("KB_QUEUES", "4"),
           _os.environ.get("KB_NOBAR", "1"), _os.environ.get("KB_REPBAR", "1"),
           _os.environ.get("KB_SCRATCH", ""), _os.environ.get("KB_SHARED", "1"),
           _os.environ.get("KB_T2COPY", "1"), _os.environ.get("KB_DEEP", "0"),
           CALL)
    if key not in _kernel_cache:
        nc = _build(K1, K2)
        _kernel_cache[key] = (nc, _make_runner(nc))
    nc, (fn, in_names, out_names, zero_outs, mesh) = _kernel_cache[key]

    sh = NamedSharding(mesh, PartitionSpec("core"))
    concat_in = [
        np.concatenate([np.asarray(in_maps[c][nm]) for c in range(NCORES)], axis=0)
        for nm in in_names
    ]
    concat_zeros = [
        np.zeros((NCORES * z.shape[0], *z.shape[1:]), z.dtype) for z in zero_outs
    ]
    dev_in = [jax.device_put(a, sh) for a in concat_in + concat_zeros]
    out_arrs = fn(*dev_in)
    jax.block_until_ready(out_arrs)

    if _bench:
        times = []
        for _ in range(5):
            t0 = _time.perf_counter()
            out_arrs = fn(*dev_in)
            jax.block_until_ready(out_arrs)
            times.append(_time.perf_counter() - t0)
        kernel._last_times = times
        kernel._last_exec_time_ns = int(min(times) * 1e9)
    else:
        kernel._last_exec_time_ns = None
    if not hasattr(kernel, "_runners"):
        kernel._runners = {}
    kernel._runners[_os.environ.get("KB_REPEAT", "1")] = (fn, dev_in)

    outs = {nm: np.asarray(out_arrs[i]) for i, nm in enumerate(out_names)}
    Y = outs["y"].reshape(NCORES, SHARD, OUT_C).reshape(NCORES * SHARD, OUT_C)
    return Y[perm_pos[:N]].astype(np.float32)
